# revision 15
# baseline (speedup 1.0000x reference)
# Bass/Trainium2 kernel for nn_AttLayer (additive-attention pooling), V2.
#
#   uit  = tanh(x[b] @ w + b)          # [S, A]
#   ait  = exp(uit @ u) * mask[b]      # [S]
#   out  = (ait @ x[b]) / (sum(ait) + EPS)   # [D]
#
# Data-parallel over batch: 4 examples/core on 8 cores.  All matmuls in
# float32r (TF32-ish rounding, 1 cyc/row at N>=256).
import numpy as np

import concourse.bass as bass
import concourse.tile as tile
from concourse import bacc, mybir
from concourse.bass_utils import run_bass_kernel_spmd

EPS = 1e-7
B, S, D, A = 32, 2048, 1024, 256
N_CORES = 8
B_LOC = B // N_CORES
S_CHUNK = 128
SLAB = 512
N_SLAB = S // SLAB            # 4
C_PER_SLAB = SLAB // S_CHUNK  # 4
DK = D // 128                 # 8
AC = A // 128                 # 2

F32 = mybir.dt.float32
F32R = mybir.dt.float32r
MMDT = F32R
BF16 = mybir.dt.bfloat16


def build_nc(n_b=B_LOC, n_t=N_SLAB, loop_reps=0):
    nc = bacc.Bacc(None)

    x_d = nc.dram_tensor("x", [B_LOC, S, D], MMDT, kind="ExternalInput")
    maskf_d = nc.dram_tensor("maskf", [1, B_LOC * S], F32, kind="ExternalInput")
    wtb_d = nc.dram_tensor("wtb", [128, DK * A], BF16, kind="ExternalInput")
    bcol_d = nc.dram_tensor("bcol", [128, AC], F32, kind="ExternalInput")
    ucol_d = nc.dram_tensor("ucol", [128, AC], MMDT, kind="ExternalInput")
    ident_d = nc.dram_tensor("ident", [128, 128], MMDT, kind="ExternalInput")
    ones_d = nc.dram_tensor("onescol", [128, 2], MMDT, kind="ExternalInput")
    out_d = nc.dram_tensor("out", [B_LOC, D], F32, kind="ExternalOutput")

    with tile.TileContext(nc) as tc:
        with (
            tc.tile_pool(name="const", bufs=1) as constp,
            tc.tile_pool(name="xnat", bufs=3) as xnatp,
            tc.tile_pool(name="xts", bufs=2) as xtsp,
            tc.tile_pool(name="uit", bufs=4) as uitp,
            tc.tile_pool(name="erow", bufs=2) as erowp,
            tc.tile_pool(name="eall", bufs=2) as eallp,
            tc.tile_pool(name="outp", bufs=2) as outp,
            tc.tile_pool(name="psxt", bufs=2, space=bass.MemorySpace.PSUM) as psxt,
            tc.tile_pool(name="psu", bufs=2, space=bass.MemorySpace.PSUM) as psu,
            tc.tile_pool(name="psmall", bufs=2, space=bass.MemorySpace.PSUM) as psmall,
            tc.tile_pool(name="psacc", bufs=1, space=bass.MemorySpace.PSUM) as psacc,
        ):
            ident = constp.tile([128, 128], MMDT, name="ident_sb")
            nc.sync.dma_start(ident[:], ident_d[:])
            ones = constp.tile([128, 2], MMDT, name="ones_sb")
            nc.sync.dma_start(ones[:], ones_d[:])
            onef = constp.tile([1, 1], F32, name="onef_sb")
            nc.vector.memset(onef[:], 1.0)
            wtb = constp.tile([128, DK * A], BF16, name="wtb_sb")
            nc.sync.dma_start(wtb[:], wtb_d[:])
            bcol = constp.tile([128, AC], F32, name="bcol_sb")
            nc.sync.dma_start(bcol[:], bcol_d[:])
            ucol = constp.tile([128, AC], MMDT, name="ucol_sb")
            nc.sync.dma_start(ucol[:], ucol_d[:])
            maskf = constp.tile([1, B_LOC * S], F32, name="maskf_sb")
            nc.sync.dma_start(maskf[:], maskf_d[:])

            import contextlib
            loop_cm = tc.For_i(0, loop_reps, 1) if loop_reps else contextlib.nullcontext()
            with loop_cm:
             for b in range(n_b):
                # [0:512],[512:1024]: pooled x accum; [1024:1026]: denom
                pacc = psacc.tile([1, 1024], F32, name="pacc")
                # one e column per s-chunk of this batch
                eall = eallp.tile([128, C_PER_SLAB * n_t], MMDT, name="eall")
                pending = []

                def emit_pool(t_, xn_):
                    for c in range(C_PER_SLAB):
                        cc = t_ * C_PER_SLAB + c
                        first = t_ == 0 and c == 0
                        last = t_ == n_t - 1 and c == C_PER_SLAB - 1
                        ecr = eall[:, cc : cc + 1]
                        nc.tensor.matmul(
                            pacc[0:1, 0:512], ecr, xn_[:, c, 0:512],
                            start=first, stop=last,
                        )
                        nc.tensor.matmul(
                            pacc[0:1, 512:1024], ecr, xn_[:, c, 512:1024],
                            start=first, stop=last,
                        )

                for t in range(n_t):
                    # one 2 MiB DMA per slab: [s=128 parts, (chunk, d) free]
                    xn = xnatp.tile([128, C_PER_SLAB, D], MMDT, name="xn", tag="xn")
                    s0 = t * SLAB
                    nc.sync.dma_start(
                        xn[:],
                        x_d[b, s0 : s0 + SLAB, :].rearrange("(c p) d -> p c d", p=128),
                    )

                    # transpose slab -> x_T [d parts, s free], dk-grouped
                    xts = xtsp.tile([128, DK * SLAB], BF16, name="xts")
                    for dk in range(DK):
                        pt = psxt.tile([128, SLAB], MMDT, name="pt", tag="pt")
                        for c in range(C_PER_SLAB):
                            nc.tensor.transpose(
                                pt[:, c * 128 : (c + 1) * 128],
                                xn[:, c, dk * 128 : (dk + 1) * 128],
                                ident[:],
                            )
                        eng = nc.scalar if dk % 2 == 0 else nc.vector
                        if dk % 2 == 0:
                            nc.scalar.copy(xts[:, dk * SLAB : (dk + 1) * SLAB], pt[:])
                        else:
                            nc.vector.tensor_copy(
                                xts[:, dk * SLAB : (dk + 1) * SLAB], pt[:]
                            )

                    # main matmul + tanh; uit [a parts, s free]
                    uits = []
                    for ac in range(AC):
                        pu = psu.tile([128, SLAB], F32, name="pu", tag="pu")
                        for dk in range(DK):
                            nc.tensor.matmul(
                                pu[:],
                                wtb[:, dk * A + ac * 128 : dk * A + (ac + 1) * 128],
                                xts[:, dk * SLAB : (dk + 1) * SLAB],
                                start=(dk == 0),
                                stop=(dk == DK - 1),
                            )
                        ui = uitp.tile([128, SLAB], MMDT, name="ui", tag="uit")
                        nc.scalar.activation(
                            ui[:],
                            pu[:],
                            mybir.ActivationFunctionType.Tanh,
                            bias=bcol[:, ac : ac + 1],
                        )
                        uits.append(ui)

                    # scores -> ait [1, SLAB] -> e_row -> e columns
                    psc = psmall.tile([1, SLAB], F32, name="psc", tag="psm")
                    for ac in range(AC):
                        nc.tensor.matmul(
                            psc[:],
                            ucol[:, ac : ac + 1],
                            uits[ac][:],
                            start=(ac == 0),
                            stop=(ac == AC - 1),
                        )
                    erow = erowp.tile([1, SLAB], F32, name="erow")
                    nc.scalar.activation(
                        erow[:], psc[:], mybir.ActivationFunctionType.Exp
                    )
                    nc.vector.tensor_mul(
                        erow[:],
                        erow[:],
                        maskf[0:1, b * S + t * SLAB : b * S + (t + 1) * SLAB],
                    )

                    for c in range(C_PER_SLAB):
                        pe = psmall.tile([128, 1], F32, name="pe", tag="psm")
                        nc.tensor.transpose(
                            pe[:], erow[0:1, c * 128 : (c + 1) * 128], onef[:]
                        )
                        cc = t * C_PER_SLAB + c
                        nc.scalar.copy(eall[:, cc : cc + 1], pe[:])

                    # pooling for the PREVIOUS slab (its e columns are ready,
                    # so these matmuls never make PE wait on this slab's
                    # ACT/DVE chain)
                    pending.append((t, xn))
                    if len(pending) > 1:
                        pt_, xn_ = pending.pop(0)
                        emit_pool(pt_, xn_)

                while pending:
                    pt_, xn_ = pending.pop(0)
                    emit_pool(pt_, xn_)

                # denominator: sum all e columns, then one matmul
                ered = outp.tile([128, 1], MMDT, name="ered", tag="ered")
                with nc.allow_low_precision(reason="16-wide sum, f32r is plenty"):
                    nc.vector.tensor_reduce(
                        ered[:], eall[:], mybir.AxisListType.X, mybir.AluOpType.add
                    )
                pden = psmall.tile([1, 2], F32, name="pden", tag="psm")
                nc.tensor.matmul(pden[:], ered[:], ones[:], start=True, stop=True)

                den = outp.tile([1, 1], F32, name="den", tag="den")
                nc.vector.tensor_scalar_add(den[:], pden[0:1, 0:1], EPS)
                rec = outp.tile([1, 1], F32, name="rec", tag="rec")
                nc.vector.reciprocal(rec[:], den[:])
                orow = outp.tile([1, D], F32, name="orow", tag="orow")
                nc.vector.tensor_scalar_mul(orow[0:1, 0:512], pacc[0:1, 0:512], rec[:])
                nc.vector.tensor_scalar_mul(
                    orow[0:1, 512:1024], pacc[0:1, 512:1024], rec[:]
                )
                nc.sync.dma_start(out_d[b : b + 1, :], orow[0:1, :])

    nc.compile()
    return nc


_NC_CACHE = None


def _get_nc():
    global _NC_CACHE
    if _NC_CACHE is None:
        _NC_CACHE = build_nc()
    return _NC_CACHE


def _prep_inputs(x, mask, w, b, u):
    x = np.asarray(x, dtype=np.float32)
    maskf = np.asarray(mask).astype(np.float32)
    w = np.asarray(w, dtype=np.float32)
    b = np.asarray(b, dtype=np.float32)
    u = np.asarray(u, dtype=np.float32)

    wt = w.reshape(DK, 128, A).transpose(1, 0, 2).reshape(128, DK * A)
    wt = np.ascontiguousarray(wt)
    import ml_dtypes
    wtb = wt.astype(ml_dtypes.bfloat16)
    bcol = np.ascontiguousarray(b.reshape(AC, 128).T)
    ucol = np.ascontiguousarray(u[:, 0].reshape(AC, 128).T)
    ident = np.eye(128, dtype=np.float32)
    onescol = np.ones((128, 2), dtype=np.float32)

    in_maps = []
    for core in range(N_CORES):
        b0 = core * B_LOC
        in_maps.append(
            {
                "x": np.ascontiguousarray(x[b0 : b0 + B_LOC]),
                "maskf": np.ascontiguousarray(
                    maskf[b0 : b0 + B_LOC].reshape(1, B_LOC * S)
                ),
                "wtb": wtb,
                "bcol": bcol,
                "ucol": ucol,
                "ident": ident,
                "onescol": onescol,
            }
        )
    return in_maps


def kernel(x, mask, w, b, u, **_kw):
    nc = _get_nc()
    in_maps = _prep_inputs(x, mask, w, b, u)
    res = run_bass_kernel_spmd(nc, in_maps, list(range(N_CORES)))
    return np.concatenate([res.results[i]["out"] for i in range(N_CORES)], axis=0)
